# revision 2
# baseline (speedup 1.0000x reference)
"""Enc-Dec MultiHeadAttention Bass/Trainium2 kernel, 8-core SPMD.

Problem: B=4, S=2048, EMB=1024, HEADS=16 (d_head=64).
  q = x_dec @ Wq.T + bq ; k = x_dec @ Wk.T + bk ; v = x_enc @ Wv.T + bv
  out = softmax(q k^T / sqrt(EMB)) v  -> @ Wp.T + bp

Sharding: core c handles batch b = c % 4 and head-group g = c // 4
(8 heads = 512 features per group).  Each core computes the attention
output for its (batch, head-group) and the partial output projection
against Wp[:, g*512:(g+1)*512]; the host sums the two partials per batch
and adds bp (standard tensor-parallel gather).

Device-side layout choices (all matmuls contract over the partition dim):
  - q/k are produced feature-major (qT/kT [feat, seq]) so attention
    energies come out k-major: eT[k, q] = kT_h.T-contraction with qT_h
    (K = d_head = 64, two heads packed in the 128-row array via
    tile_position row tiling).
  - exp runs on ScalarE directly from PSUM ([128, 1024] per op covering
    both heads) with scale=1/32 folded into the activation.
  - v is produced seq-major with a ones-column appended per head
    (v_aug[k, 65]); PV matmul (v_aug stationary, attT moving) yields
    outT_unnorm rows 0..63 plus the softmax denominator in row 64 of the
    same PSUM accumulation.
  - normalization: reciprocal of denominators on VectorE, broadcast
    across 128 partitions via a tiny K=2 ones-matmul (operands viewed as
    float32r: 1 cycle/row instead of fp32's 4), one tensor_tensor
    multiply.
  - weights arrive host-pre-tiled to the exact SBUF layout so each weight
    DMA moves 8KB-contiguous runs per partition (~2x the descriptor
    efficiency of per-e-tile 1KB runs).
"""

import numpy as np
import ml_dtypes
from contextlib import ExitStack

EMB = 1024
S = 2048
B = 4
NCORES = 8
D = 64
HPC = 8            # heads per core
FG = HPC * D       # 512 features per head-group
NPAIRS = HPC // 2  # head pairs per core (row-tiled in the PE array)
CH = 512           # q-chunk width
NCH = S // CH      # 4
ET = EMB // 128    # 8 contraction tiles for the projections
ST = S // 128      # 16 seq tiles
SCALE = float(EMB) ** -0.5  # 1/32

_cache = {}


def _mha_body(tc, ctx, xdT, xeT, wqT, wkT, wvT, wpT, bq, bk, bv, out):
    import concourse.bass as bass
    from concourse import mybir

    nc = tc.nc
    f32 = mybir.dt.float32
    bf16 = mybir.dt.bfloat16
    Exp = mybir.ActivationFunctionType.Exp
    mult = mybir.AluOpType.mult

    wpool = ctx.enter_context(tc.tile_pool(name="weights", bufs=1))
    vpool = ctx.enter_context(tc.tile_pool(name="vaug", bufs=1))
    xpool = ctx.enter_context(tc.tile_pool(name="xstream", bufs=1))
    qkpool = ctx.enter_context(tc.tile_pool(name="qk", bufs=2))
    attpool = ctx.enter_context(tc.tile_pool(name="att", bufs=2))
    opool = ctx.enter_context(tc.tile_pool(name="outu", bufs=1))
    dnpool = ctx.enter_context(tc.tile_pool(name="dn", bufs=1))
    stpool = ctx.enter_context(tc.tile_pool(name="stage", bufs=3))
    ps_mm = ctx.enter_context(tc.tile_pool(name="psmm", bufs=2, space="PSUM"))
    ps_et = ctx.enter_context(tc.tile_pool(name="pset", bufs=2, space="PSUM"))
    ps_pv = ctx.enter_context(tc.tile_pool(name="pspv", bufs=1, space="PSUM"))

    # --- weights / constants -------------------------------------------------
    # emission order matters for DMA queue scheduling: the v-phase consumes
    # xe/wv e-tile by e-tile, so those DMAs go first, interleaved.
    # PE warmup first — no DMA dependencies, so it runs at t~1us: ~5us of
    # dense matmuls flip the HAM clock gate to 2.4GHz before the real work
    # (otherwise the whole v+proj startup runs at 1.2GHz)
    warm_t = wpool.tile([128, CH], bf16, tag="warm")
    nc.vector.memset(warm_t[:], 0.5)
    for i in range(8):
        pw = ps_mm.tile([128, CH], f32, tag="mm", name="pw")
        nc.tensor.matmul(pw[:], warm_t[:, 0:128], warm_t[:], start=True, stop=True)

    # DMA queue order = emission order: tiny bias tensors first (the v-bias
    # matmul would otherwise stall the in-order PE stream on them), then wv +
    # kt-major xe (the V projection starts as soon as kt-slice 0 lands),
    # then the pair-0 projection inputs, which land while v computes.
    bq_sb = wpool.tile([128, FG // 128], f32, tag="bq")
    nc.sync.dma_start(bq_sb[:], bq.rearrange("(o p) -> p o", p=128))
    bk_sb = wpool.tile([128, FG // 128], f32, tag="bk")
    nc.sync.dma_start(bk_sb[:], bk.rearrange("(o p) -> p o", p=128))
    bv_f = wpool.tile([1, FG], f32, tag="bvf")
    nc.sync.dma_start(bv_f[:], bv.rearrange("(o f) -> o f", o=1))
    bv_sb = wpool.tile([1, FG], bf16, tag="bv")
    nc.vector.tensor_copy(bv_sb[:], bv_f[:])

    # x_enc arrives kt-major (host pre-tiled): xe_sb[p, kt, e, c] =
    # x_enc[kt*128 + c, e*128 + p] — the V projection's kt-step needs only
    # its own 256KB slice, so V compute overlaps the x_enc DMA stream
    xe_sb = xpool.tile([128, ST, ET, 128], bf16, tag="x")
    xeKr = xeT.rearrange("t p (e c) -> t p e c", c=128)
    wv_sb = wpool.tile([128, ET, FG], bf16, tag="wv")
    nc.sync.dma_start(wv_sb[:], wvT)
    for kt in range(ST):
        nc.sync.dma_start(xe_sb[:, kt], xeKr[kt])
    wq_sb = wpool.tile([128, ET, FG], bf16, tag="wq")
    nc.sync.dma_start(wq_sb[:], wqT)
    wk_sb = wpool.tile([128, ET, FG], bf16, tag="wk")
    nc.sync.dma_start(wk_sb[:], wkT)
    xd_sb = xpool.tile([128, ET, S], bf16, tag="xd")
    xdTr = xdT.rearrange("(o p) s -> p o s", p=128)
    for e in range(ET):
        nc.sync.dma_start(xd_sb[:, e], xdTr[:, e])
    wp_sb = wpool.tile([128, FG // 128, EMB], bf16, tag="wp")
    nc.sync.dma_start(wp_sb[:], wpT)

    ones_v = wpool.tile([1, 128], bf16, tag="ones_v")
    nc.vector.memset(ones_v[:], 1.0)
    # ones2[0, 0:64] = 1, ones2[1, 64:128] = 1: broadcasts the two heads'
    # per-q reciprocals onto the pair's 128 feature partitions.
    ones2 = wpool.tile([2, 128], f32, tag="ones2")
    nc.vector.memset(ones2[:], 0.0)
    nc.vector.memset(ones2[0:1, 0:64], 1.0)
    # engine ops can't start at partition 1; DMA is partition-free
    nc.sync.dma_start(ones2[1:2, 64:128], ones2[0:1, 0:64])

    # --- V projection buffers (emitted below, after the cascade defs) -------
    v_aug = vpool.tile([128, ST, HPC * 65], bf16, tag="vaug")
    va4 = v_aug.rearrange("p s (h f) -> p s h f", f=65)
    nc.vector.memset(va4[:, :, :, 64:65], 1.0)

    # --- persistent attention-output + denominator buffers ------------------
    # outu[p, pair, s]: partitions = pair-local features (64*j + d)
    outu = opool.tile([128, NPAIRS, S], bf16, tag="outu")
    # one row per (head, chunk) at partition 32*pair + 4*j + ch, so each
    # pair's rows start 32-aligned (engine ops need aligned start partitions)
    dn_sp = dnpool.tile([128, CH], f32, tag="dn")
    rc_sp = dnpool.tile([128, CH], f32, tag="rc")
    # only 8 rows per 32-row block get real denominators; init the rest so
    # the batched reciprocal over all 128 partitions reads defined values
    nc.vector.memset(dn_sp[:], 1.0)

    def normalize_chunk(pair, ch):
        cs = slice(ch * CH, (ch + 1) * CH)
        rc_t = dnpool.tile([2, CH], f32, tag="rct", bufs=2, name="rc_t")
        for j in range(2):
            r = 32 * pair + 4 * j + ch
            nc.sync.dma_start(rc_t[j:j + 1, :], rc_sp[r:r + 1, :])
        pb = ps_mm.tile([128, CH], f32, tag="mm", name="pb")
        f32r = mybir.dt.float32r
        nc.tensor.matmul(pb[:], ones2[:].bitcast(f32r), rc_t[:].bitcast(f32r),
                         start=True, stop=True)
        nc.vector.tensor_tensor(outu[:, pair, cs], outu[:, pair, cs], pb[:], mult)

    norm_q = []

    def proj_piece(ppair, which, ch, qT_, kT_):
        """One 512-col chunk of the q or k projection for pair `ppair`."""
        fs_ = slice(ppair * 128, (ppair + 1) * 128)
        cs_ = slice(ch * CH, (ch + 1) * CH)
        w_sb, b_sb, dst = ((wq_sb, bq_sb, qT_) if which == "q"
                          else (wk_sb, bk_sb, kT_))
        pp = ps_mm.tile([128, CH], f32, tag="mm", name="pp")
        for e in range(ET):
            nc.tensor.matmul(
                pp[:], w_sb[:, e, fs_], xd_sb[:, e, cs_],
                start=(e == 0), stop=(e == ET - 1),
            )
        nc.vector.tensor_scalar_add(dst[:, cs_], pp[:], b_sb[:, ppair:ppair + 1])

    # --- projection cascade -------------------------------------------------
    # q is consumed chunk-locally (one piece ahead suffices); k is consumed
    # across the full key range by every chunk, so the next pair's k pieces
    # spread across the current pair's four chunks.
    qk_tiles = {}

    def get_qk(p):
        if p not in qk_tiles:
            q_ = qkpool.tile([128, S], bf16, tag="qT", name=f"qT{p}")
            k_ = qkpool.tile([128, S], bf16, tag="kT", name=f"kT{p}")
            qk_tiles[p] = (q_, k_)
        return qk_tiles[p]

    def piece(p_, w_, c_):
        q_, k_ = get_qk(p_)
        proj_piece(p_, w_, c_, q_, k_)

    # --- V projection (seq-major, with ones column per head) ----------------
    # full-width (all heads, N=512): DMA-paced and PE-efficient; pair-0's
    # projection pieces ride the tail of this stream (their inputs are the
    # last DMAs to land anyway)
    for kt in range(ST):
        pvf = ps_mm.tile([128, FG], f32, tag="mm", name="pvf")
        for e in range(ET):
            nc.tensor.matmul(
                pvf[:], xe_sb[:, kt, e], wv_sb[:, e, :],
                start=(e == 0), stop=False,
            )
        nc.tensor.matmul(pvf[:], ones_v[:], bv_sb[:], start=False, stop=True)
        nc.vector.tensor_copy(
            va4[:, kt, :, 0:64], pvf.rearrange("p (h f) -> p h f", f=64))

    piece(0, "q", 0)
    for c in range(NCH):
        piece(0, "k", c)

    # the previous chunk's PV tail + copies carry across the chunk boundary
    # (run inside the next chunk's first blocks) so the new chunk's QK and
    # exp stream issue immediately after the old chunk's last exp.
    #
    # kt-steps run in PAIRS: two energy row-tile pairs back-to-back (the
    # second pair's LDWEIGHTS hides in the background weight buffer — same
    # PE tiling mode), then four PV matmuls back-to-back (same: only the
    # first pays the mode-switch + LDW serialization, ~120ns; the rest run
    # at the 217ns streaming floor).  attT bufs=6 covers the pv lag of 4.
    carry = []

    for pair in range(NPAIRS):
        qT, kT = get_qk(pair)

        # --- attention for the pair -----------------------------------------
        for ch in range(NCH):
            cs = slice(ch * CH, (ch + 1) * CH)
            ppv = [ps_pv.tile([65, CH], f32, tag=f"pv{j}", name=f"ppv{j}")
                   for j in range(2)]
            attks = {}

            def pv_step(kt, ppv_=ppv, attks_=attks, pair_=pair):
                attk = attks_.pop(kt)
                for j in range(2):
                    h = 2 * pair_ + j
                    nc.tensor.matmul(
                        ppv_[j][:], v_aug[:, kt, h * 65:(h + 1) * 65],
                        attk[:, j * CH:(j + 1) * CH],
                        start=(kt == 0), stop=(kt == ST - 1),
                    )

            def tail_copies(pair_=pair, ch_=ch, cs_=cs, ppv_=ppv):
                for j in range(2):
                    nc.vector.tensor_copy(
                        outu[64 * j:64 * (j + 1), pair_, cs_], ppv_[j][0:64, :])
                    # denominator row: DVE stays in lane 64 (engines can't
                    # cross partitions); a tiny DMA moves it to its dn_sp row
                    dn_st = dnpool.tile([128, CH], f32, tag="dnstage", bufs=2)
                    nc.vector.tensor_copy(dn_st[64:65, :], ppv_[j][64:65, :])
                    r = 32 * pair_ + 4 * j + ch_
                    nc.sync.dma_start(dn_sp[r:r + 1, :], dn_st[64:65, :])
                if pair_ == NPAIRS - 1 and ch_ < NCH - 1:
                    nc.vector.reciprocal(rc_sp[96:128, :], dn_sp[96:128, :])
                    norm_q.append((pair_, ch_))

            def energy(kt):
                ks = slice(kt * 128, (kt + 1) * 128)
                eT = ps_et.tile([128, 2 * CH], f32, tag="eT")
                nc.tensor.matmul(
                    eT[:, 0:CH], kT[0:64, ks], qT[0:64, cs],
                    start=True, stop=True,
                )
                nc.tensor.matmul(
                    eT[:, CH:2 * CH], kT[64:128, ks], qT[64:128, cs],
                    start=True, stop=True,
                )
                attk = attpool.tile([128, 2 * CH], bf16, tag="attT", bufs=6)
                nc.scalar.activation(attk[:], eT[:], Exp, scale=SCALE)
                attks[kt] = attk

            for t2 in range(ST // 2):
                kt0, kt1 = 2 * t2, 2 * t2 + 1
                energy(kt0)
                energy(kt1)
                if t2 < len(carry):
                    carry[t2]()
                # PV runs 4 k-tiles behind exp, two kt-steps per block; the
                # last four PVs + copies of this chunk run inside the NEXT
                # chunk's first blocks
                if t2 >= 2:
                    pv_step(kt0 - 4)
                    pv_step(kt1 - 4)
                # the projection cascade
                if t2 == 0 and ch < NCH - 1:
                    piece(pair, "q", ch + 1)
                if t2 == 4 and pair < NPAIRS - 1:
                    piece(pair + 1, "k", ch)
                if t2 == 2 and ch == NCH - 1 and pair < NPAIRS - 1:
                    piece(pair + 1, "q", 0)
                # pair-3 entry: batch-reciprocal pairs 0..2 denominators once
                # their dn rows (carried from pair2-ch3) have been emitted
                if pair == NPAIRS - 1 and ch == 0 and t2 == 2:
                    nc.vector.reciprocal(rc_sp[0:96, :], dn_sp[0:96, :])
                    # chunk-major so outproj's low q-tiles unblock earliest
                    norm_q.extend((p_, c_) for c_ in range(NCH)
                                  for p_ in range(NPAIRS - 1))
                # ... and the deferred normalizations during the last pair
                if norm_q and (ch > 0 or
                               (pair == NPAIRS - 1 and t2 >= 3)):
                    normalize_chunk(*norm_q.pop(0))
            if (pair, ch) == (NPAIRS - 1, NCH - 1):
                pv_step(ST - 4)
                pv_step(ST - 3)
                pv_step(ST - 2)
                pv_step(ST - 1)
                tail_copies()
                carry = []
            else:
                carry = [lambda p=pv_step: (p(ST - 4), p(ST - 3)),
                         lambda p=pv_step: (p(ST - 2), p(ST - 1)),
                         tail_copies]
    # --- tail: outproj for chunks 0..2 first (they don't need the last
    # chunk's normalization), the (3,3) normalize chain resolves meanwhile --
    nc.vector.reciprocal(rc_sp[96:128, :], dn_sp[96:128, :])
    for item in norm_q:  # any leftovers for pairs 0..2 / (3, ch<3)
        normalize_chunk(*item)
    norm_q.clear()

    def outproj(qt):
        # both 512-wide output halves of one q-tile share a 2-bank PSUM
        # tile: 8 matmuls, then ONE 1024-wide copy + DMA
        qs = slice(qt * 128, (qt + 1) * 128)
        po = ps_et.tile([128, 2 * CH], f32, tag="eT", name="po")
        for ot in range(EMB // CH):
            os_ = slice(ot * CH, (ot + 1) * CH)
            for pk in range(NPAIRS):
                nc.tensor.matmul(
                    po[:, os_], outu[:, pk, qs], wp_sb[:, pk, os_],
                    start=(pk == 0), stop=(pk == NPAIRS - 1),
                )
        so = stpool.tile([128, 2 * CH], f32, tag="so")
        if qt % 2 == 0:  # split tail copies across the two idle engines
            nc.scalar.copy(so[:], po[:])
        else:
            nc.vector.tensor_copy(so[:], po[:])
        nc.sync.dma_start(out[qs, :], so[:])

    for qt in range(3 * ST // 4):
        outproj(qt)
    normalize_chunk(NPAIRS - 1, NCH - 1)
    for qt in range(3 * ST // 4, ST):
        outproj(qt)


def build():
    """Build + compile the per-core Bass program (cached)."""
    if "nc" in _cache:
        return _cache["nc"]
    import concourse.tile as tile
    from concourse import bacc, mybir

    f32 = mybir.dt.float32
    bf16 = mybir.dt.bfloat16
    nc = bacc.Bacc("TRN2", target_bir_lowering=False, debug=False,
                   num_devices=NCORES)
    xdT = nc.dram_tensor("xdT", (EMB, S), bf16, kind="ExternalInput").ap()
    xeT = nc.dram_tensor("xeT", (ST, 128, EMB), bf16, kind="ExternalInput").ap()
    wqT = nc.dram_tensor("wqT", (128, ET, FG), bf16, kind="ExternalInput").ap()
    wkT = nc.dram_tensor("wkT", (128, ET, FG), bf16, kind="ExternalInput").ap()
    wvT = nc.dram_tensor("wvT", (128, ET, FG), bf16, kind="ExternalInput").ap()
    wpT = nc.dram_tensor("wpT", (128, FG // 128, EMB), bf16, kind="ExternalInput").ap()
    bq = nc.dram_tensor("bq", (FG,), f32, kind="ExternalInput").ap()
    bk = nc.dram_tensor("bk", (FG,), f32, kind="ExternalInput").ap()
    bv = nc.dram_tensor("bv", (FG,), f32, kind="ExternalInput").ap()
    out = nc.dram_tensor("out", (S, EMB), f32, kind="ExternalOutput").ap()

    with tile.TileContext(nc) as tc:
        with ExitStack() as ctx:
            _mha_body(tc, ctx, xdT, xeT, wqT, wkT, wvT, wpT, bq, bk, bv, out)
    nc.compile()
    _cache["nc"] = nc
    return nc


def make_in_maps(x_enc, x_dec, Wq, bq, Wk, bk, Wv, bv, Wp):
    """Host-side sharding: per-core input dict for core c = (g = c//4, b = c%4)."""
    bf = ml_dtypes.bfloat16

    def tile_w(w):     # [EMB_in, F] -> [128, ET_in, F]
        ei, f = w.shape
        return np.ascontiguousarray(
            w.reshape(ei // 128, 128, f).transpose(1, 0, 2)).astype(bf)

    in_maps = []
    xdTs = [np.ascontiguousarray(x_dec[b].T).astype(bf) for b in range(B)]
    # kt-major x_enc: xeT[kt, p, e*128 + c] = x_enc[kt*128 + c, e*128 + p]
    xeTs = []
    for b in range(B):
        blocks = x_enc[b].reshape(ST, 128, ET, 128)         # [kt, c, e, p]
        xeTs.append(np.ascontiguousarray(
            blocks.transpose(0, 3, 2, 1).reshape(ST, 128, EMB)).astype(bf))
    for c in range(NCORES):
        g, b = divmod(c, B)
        gs = slice(g * FG, (g + 1) * FG)
        in_maps.append({
            "xdT": xdTs[b],
            "xeT": xeTs[b],
            "wqT": tile_w(np.ascontiguousarray(Wq[gs].T)),
            "wkT": tile_w(np.ascontiguousarray(Wk[gs].T)),
            "wvT": tile_w(np.ascontiguousarray(Wv[gs].T)),
            "wpT": tile_w(np.ascontiguousarray(Wp[:, gs].T)),
            "bq": np.ascontiguousarray(bq[gs]).astype(np.float32),
            "bk": np.ascontiguousarray(bk[gs]).astype(np.float32),
            "bv": np.ascontiguousarray(bv[gs]).astype(np.float32),
        })
    return in_maps


def kernel(x_enc, x_dec, Wq, bq, Wk, bk, Wv, bv, Wp, bp):
    from concourse.bass_utils import run_bass_kernel_spmd

    x_enc = np.asarray(x_enc, dtype=np.float32)
    x_dec = np.asarray(x_dec, dtype=np.float32)
    nc = build()
    in_maps = make_in_maps(np.asarray(x_enc), np.asarray(x_dec),
                           np.asarray(Wq), np.asarray(bq), np.asarray(Wk),
                           np.asarray(bk), np.asarray(Wv), np.asarray(bv),
                           np.asarray(Wp))
    res = run_bass_kernel_spmd(nc, in_maps, core_ids=list(range(NCORES)))
    out = np.empty((B, S, EMB), dtype=np.float32)
    bp32 = np.asarray(bp, dtype=np.float32)
    for b in range(B):
        out[b] = res.results[b]["out"] + res.results[b + B]["out"] + bp32
    return out



# revision 14
# speedup vs baseline: 1.0191x; 1.0191x over previous
"""Enc-Dec MultiHeadAttention Bass/Trainium2 kernel, 8-core SPMD.

Problem: B=4, S=2048, EMB=1024, HEADS=16 (d_head=64).
  q = x_dec @ Wq.T + bq ; k = x_dec @ Wk.T + bk ; v = x_enc @ Wv.T + bv
  out = softmax(q k^T / sqrt(EMB)) v  -> @ Wp.T + bp

Sharding: core c handles batch b = c % 4 and head-group g = c // 4
(8 heads = 512 features per group).  Each core computes the attention
output for its (batch, head-group) and the partial output projection
against Wp[:, g*512:(g+1)*512]; the host sums the two partials per batch
and adds bp (standard tensor-parallel gather).

Device-side layout choices (all matmuls contract over the partition dim):
  - q/k are produced feature-major (qT/kT [feat, seq]) so attention
    energies come out k-major: eT[k, q] = kT_h.T-contraction with qT_h
    (K = d_head = 64, two heads packed in the 128-row array via
    tile_position row tiling).
  - exp runs on ScalarE directly from PSUM ([128, 1024] per op covering
    both heads) with scale=1/32 folded into the activation.
  - v is produced seq-major with a ones-column appended per head
    (v_aug[k, 65]); PV matmul (v_aug stationary, attT moving) yields
    outT_unnorm rows 0..63 plus the softmax denominator in row 64 of the
    same PSUM accumulation.
  - normalization: reciprocal of denominators on VectorE, broadcast
    across 128 partitions via a tiny K=2 ones-matmul (operands viewed as
    float32r: 1 cycle/row instead of fp32's 4), one tensor_tensor
    multiply.
  - weights arrive host-pre-tiled to the exact SBUF layout so each weight
    DMA moves 8KB-contiguous runs per partition (~2x the descriptor
    efficiency of per-e-tile 1KB runs).
"""

import numpy as np
import ml_dtypes
from contextlib import ExitStack

EMB = 1024
S = 2048
B = 4
NCORES = 8
D = 64
HPC = 8            # heads per core
FG = HPC * D       # 512 features per head-group
NPAIRS = HPC // 2  # head pairs per core (row-tiled in the PE array)
CH = 512           # q-chunk width
NCH = S // CH      # 4
ET = EMB // 128    # 8 contraction tiles for the projections
ST = S // 128      # 16 seq tiles
SCALE = float(EMB) ** -0.5  # 1/32

_cache = {}


def _mha_body(tc, ctx, xdT, xeT, wqT, wkT, wvT, wpT, bq, bk, bv, out):
    import concourse.bass as bass
    from concourse import mybir

    nc = tc.nc
    f32 = mybir.dt.float32
    bf16 = mybir.dt.bfloat16
    Exp = mybir.ActivationFunctionType.Exp
    mult = mybir.AluOpType.mult

    wpool = ctx.enter_context(tc.tile_pool(name="weights", bufs=1))
    vpool = ctx.enter_context(tc.tile_pool(name="vaug", bufs=1))
    xpool = ctx.enter_context(tc.tile_pool(name="xstream", bufs=1))
    qkpool = ctx.enter_context(tc.tile_pool(name="qk", bufs=2))
    attpool = ctx.enter_context(tc.tile_pool(name="att", bufs=2))
    opool = ctx.enter_context(tc.tile_pool(name="outu", bufs=1))
    dnpool = ctx.enter_context(tc.tile_pool(name="dn", bufs=1))
    stpool = ctx.enter_context(tc.tile_pool(name="stage", bufs=3))
    ps_mm = ctx.enter_context(tc.tile_pool(name="psmm", bufs=2, space="PSUM"))
    ps_et = ctx.enter_context(tc.tile_pool(name="pset", bufs=2, space="PSUM"))
    ps_pv = ctx.enter_context(tc.tile_pool(name="pspv", bufs=1, space="PSUM"))

    # --- weights / constants -------------------------------------------------
    # emission order matters for DMA queue scheduling: the v-phase consumes
    # xe/wv e-tile by e-tile, so those DMAs go first, interleaved.
    # PE warmup first — no DMA dependencies, so it runs at t~1us: ~5us of
    # dense matmuls flip the HAM clock gate to 2.4GHz before the real work
    # (otherwise the whole v+proj startup runs at 1.2GHz)
    warm_t = wpool.tile([128, CH], bf16, tag="warm")
    nc.vector.memset(warm_t[:], 0.5)
    for i in range(14):
        pw = ps_mm.tile([128, CH], f32, tag="mm", name="pw")
        nc.tensor.matmul(pw[:], warm_t[:, 0:128], warm_t[:], start=True, stop=True)

    # DMA queue order = emission order: tiny bias tensors first (the v-bias
    # matmul would otherwise stall the in-order PE stream on them), then wv +
    # kt-major xe (the V projection starts as soon as kt-slice 0 lands),
    # then the pair-0 projection inputs, which land while v computes.
    bq_sb = wpool.tile([128, FG // 128], f32, tag="bq")
    nc.sync.dma_start(bq_sb[:], bq.rearrange("(o p) -> p o", p=128))
    bk_sb = wpool.tile([128, FG // 128], f32, tag="bk")
    nc.sync.dma_start(bk_sb[:], bk.rearrange("(o p) -> p o", p=128))
    # bv arrives host pre-tiled [128, NPAIRS]: partition 64j+d, column pair.
    # It is added AFTER normalization (exactly equivalent: sum(att*(v+bv))/dn
    # == sum(att*v)/dn + bv), which kills the per-kt K=1 bias matmul.
    bv_sb = wpool.tile([128, NPAIRS], f32, tag="bv")
    nc.sync.dma_start(bv_sb[:], bv)

    # x_enc arrives kt-major (host pre-tiled): xe_sb[p, kt, e, c] =
    # x_enc[kt*128 + c, e*128 + p] — the V projection's kt-step needs only
    # its own 256KB slice, so V compute overlaps the x_enc DMA stream
    xe_sb = xpool.tile([128, ST, ET, 128], bf16, tag="x")
    xeKr = xeT.rearrange("t p (e c) -> t p e c", c=128)
    wv_sb = wpool.tile([128, ET, FG], bf16, tag="wv")
    nc.sync.dma_start(wv_sb[:], wvT)
    for kt in range(ST):
        nc.sync.dma_start(xe_sb[:, kt], xeKr[kt])
    wq_sb = wpool.tile([128, ET, FG], bf16, tag="wq")
    nc.sync.dma_start(wq_sb[:], wqT)
    wk_sb = wpool.tile([128, ET, FG], bf16, tag="wk")
    nc.sync.dma_start(wk_sb[:], wkT)
    xd_sb = xpool.tile([128, ET, S], bf16, tag="xd")
    xdTr = xdT.rearrange("(o p) s -> p o s", p=128)
    for e in range(ET):
        nc.sync.dma_start(xd_sb[:, e], xdTr[:, e])
    wp_sb = wpool.tile([128, FG // 128, EMB], bf16, tag="wp")
    nc.sync.dma_start(wp_sb[:], wpT)

    # ones2[0, 0:64] = 1, ones2[1, 64:128] = 1: broadcasts the two heads'
    # per-q reciprocals onto the pair's 128 feature partitions.
    ones2 = wpool.tile([2, 128], bf16, tag="ones2")
    nc.vector.memset(ones2[:], 0.0)
    nc.vector.memset(ones2[0:1, 0:64], 1.0)
    # engine ops can't start at partition 1; DMA is partition-free
    nc.sync.dma_start(ones2[1:2, 64:128], ones2[0:1, 0:64])

    # --- V projection buffers (emitted below, after the cascade defs) -------
    v_aug = vpool.tile([128, ST, HPC * 65], bf16, tag="vaug")
    va4 = v_aug.rearrange("p s (h f) -> p s h f", f=65)
    nc.vector.memset(va4[:, :, :, 64:65], 1.0)

    # --- persistent attention-output + denominator buffers ------------------
    # outu[p, pair, s]: partitions = pair-local features (64*j + d)
    outu = opool.tile([128, NPAIRS, S], bf16, tag="outu")
    # one row per (head, chunk) at partition 32*pair + 4*j + ch, so each
    # pair's rows start 32-aligned (engine ops need aligned start partitions)
    dn_sp = dnpool.tile([128, CH], f32, tag="dn")
    rc_sp = dnpool.tile([128, CH], bf16, tag="rc")
    # only 8 rows per 32-row block get real denominators; init the rest so
    # the batched reciprocal over all 128 partitions reads defined values
    nc.vector.memset(dn_sp[:], 1.0)

    def normalize_chunk(pair, ch):
        cs = slice(ch * CH, (ch + 1) * CH)
        rc_t = dnpool.tile([2, CH], bf16, tag="rct", bufs=2, name="rc_t")
        for j in range(2):
            r = 32 * pair + 4 * j + ch
            nc.sync.dma_start(rc_t[j:j + 1, :], rc_sp[r:r + 1, :])
        pb = ps_mm.tile([128, CH], f32, tag="mm", name="pb")
        nc.tensor.matmul(pb[:], ones2[:], rc_t[:], start=True, stop=True)
        nc.vector.tensor_tensor(outu[:, pair, cs], outu[:, pair, cs], pb[:], mult)
        nc.vector.tensor_scalar_add(outu[:, pair, cs], outu[:, pair, cs],
                                    bv_sb[:, pair:pair + 1])

    norm_q = []

    def proj_piece(ppair, which, ch, qT_, kT_):
        """One 512-col chunk of the q or k projection for pair `ppair`."""
        fs_ = slice(ppair * 128, (ppair + 1) * 128)
        cs_ = slice(ch * CH, (ch + 1) * CH)
        w_sb, b_sb, dst = ((wq_sb, bq_sb, qT_) if which == "q"
                          else (wk_sb, bk_sb, kT_))
        pp = ps_mm.tile([128, CH], f32, tag="mm", name="pp")
        for e in range(ET):
            nc.tensor.matmul(
                pp[:], w_sb[:, e, fs_], xd_sb[:, e, cs_],
                start=(e == 0), stop=(e == ET - 1),
            )
        nc.vector.tensor_scalar_add(dst[:, cs_], pp[:], b_sb[:, ppair:ppair + 1])

    # --- projection cascade -------------------------------------------------
    # q is consumed chunk-locally (one piece ahead suffices); k is consumed
    # across the full key range by every chunk, so the next pair's k pieces
    # spread across the current pair's four chunks.
    qk_tiles = {}

    def get_qk(p):
        if p not in qk_tiles:
            q_ = qkpool.tile([128, S], bf16, tag="qT", name=f"qT{p}")
            k_ = qkpool.tile([128, S], bf16, tag="kT", name=f"kT{p}")
            qk_tiles[p] = (q_, k_)
        return qk_tiles[p]

    def piece(p_, w_, c_):
        q_, k_ = get_qk(p_)
        proj_piece(p_, w_, c_, q_, k_)

    # --- V projection (seq-major, with ones column per head) ----------------
    # full-width (all heads, N=512): DMA-paced and PE-efficient; pair-0's
    # projection pieces ride the tail of this stream (their inputs are the
    # last DMAs to land anyway)
    for kt in range(ST):
        pvf = ps_mm.tile([128, FG], f32, tag="mm", name="pvf")
        for e in range(ET):
            nc.tensor.matmul(
                pvf[:], xe_sb[:, kt, e], wv_sb[:, e, :],
                start=(e == 0), stop=(e == ET - 1),
            )
        nc.vector.tensor_copy(
            va4[:, kt, :, 0:64], pvf.rearrange("p (h f) -> p h f", f=64))

    # only chunk-0's q/k pieces run before the attention stream; k-pieces for
    # chunks 1-3 ride inside ch0 itself (chunk c's keys are first touched at
    # kt=4c, two blocks after the piece is injected)
    piece(0, "q", 0)
    piece(0, "k", 0)

    # the previous chunk's PV tail + copies carry across the chunk boundary
    # (run inside the next chunk's first blocks) so the new chunk's QK and
    # exp stream issue immediately after the old chunk's last exp.
    #
    # kt-steps run in PAIRS: two energy row-tile pairs back-to-back (the
    # second pair's LDWEIGHTS hides in the background weight buffer — same
    # PE tiling mode), then four PV matmuls back-to-back (same: only the
    # first pays the mode-switch + LDW serialization, ~120ns; the rest run
    # at the 217ns streaming floor).  attT bufs=6 covers the pv lag of 4.
    carry = []

    for pair in range(NPAIRS):
        qT, kT = get_qk(pair)

        # --- attention for the pair -----------------------------------------
        for ch in range(NCH):
            cs = slice(ch * CH, (ch + 1) * CH)
            ppv = [ps_pv.tile([65, CH], f32, tag=f"pv{j}", name=f"ppv{j}")
                   for j in range(2)]
            attks = {}

            def pv_step(kt, ppv_=ppv, attks_=attks, pair_=pair):
                attk = attks_.pop(kt)
                for j in range(2):
                    h = 2 * pair_ + j
                    nc.tensor.matmul(
                        ppv_[j][:], v_aug[:, kt, h * 65:(h + 1) * 65],
                        attk[:, j * CH:(j + 1) * CH],
                        start=(kt == 0), stop=(kt == ST - 1),
                    )

            def tail_copies(pair_=pair, ch_=ch, cs_=cs, ppv_=ppv):
                for j in range(2):
                    nc.vector.tensor_copy(
                        outu[64 * j:64 * (j + 1), pair_, cs_], ppv_[j][0:64, :])
                    # denominator row: DVE stays in lane 64 (engines can't
                    # cross partitions); a tiny DMA moves it to its dn_sp row
                    dn_st = dnpool.tile([128, CH], f32, tag="dnstage", bufs=2)
                    nc.vector.tensor_copy(dn_st[64:65, :], ppv_[j][64:65, :])
                    r = 32 * pair_ + 4 * j + ch_
                    nc.sync.dma_start(dn_sp[r:r + 1, :], dn_st[64:65, :])
                if pair_ == NPAIRS - 1 and ch_ < NCH - 1:
                    with nc.allow_low_precision(reason="bf16 softmax scale"):
                        nc.vector.reciprocal(rc_sp[96:128, :], dn_sp[96:128, :])
                    norm_q.append((pair_, ch_))

            def energy(kt):
                ks = slice(kt * 128, (kt + 1) * 128)
                eT = ps_et.tile([128, 2 * CH], f32, tag="eT")
                nc.tensor.matmul(
                    eT[:, 0:CH], kT[0:64, ks], qT[0:64, cs],
                    start=True, stop=True,
                )
                nc.tensor.matmul(
                    eT[:, CH:2 * CH], kT[64:128, ks], qT[64:128, cs],
                    start=True, stop=True,
                )
                attk = attpool.tile([128, 2 * CH], bf16, tag="attT", bufs=6)
                nc.scalar.activation(attk[:], eT[:], Exp, scale=SCALE)
                attks[kt] = attk

            for t2 in range(ST // 2):
                kt0, kt1 = 2 * t2, 2 * t2 + 1
                energy(kt0)
                energy(kt1)
                if t2 < len(carry):
                    carry[t2]()
                # PV runs 4 k-tiles behind exp, two kt-steps per block; the
                # last four PVs + copies of this chunk run inside the NEXT
                # chunk's first blocks
                if t2 >= 2:
                    pv_step(kt0 - 4)
                    pv_step(kt1 - 4)
                # the projection cascade: own k-pieces for chunks 1-3 land in
                # ch0 (chunk c's keys first touched at kt=4c); own q for the
                # next chunk mid-chunk; the next pair's (q0, k0) late in ch3
                if ch == 0 and t2 in (0, 2, 4):
                    piece(pair, "k", t2 // 2 + 1)
                if t2 == (6 if ch == 0 else 0) and ch < NCH - 1:
                    piece(pair, "q", ch + 1)
                if t2 == 2 and ch == NCH - 1 and pair < NPAIRS - 1:
                    piece(pair + 1, "q", 0)
                if t2 == 4 and ch == NCH - 1 and pair < NPAIRS - 1:
                    piece(pair + 1, "k", 0)
                # pair-3 entry: batch-reciprocal pairs 0..2 denominators once
                # their dn rows (carried from pair2-ch3) have been emitted
                if pair == NPAIRS - 1 and ch == 0 and t2 == 2:
                    with nc.allow_low_precision(reason="bf16 softmax scale"):
                        nc.vector.reciprocal(rc_sp[0:96, :], dn_sp[0:96, :])
                    # chunk-major so outproj's low q-tiles unblock earliest
                    norm_q.extend((p_, c_) for c_ in range(NCH)
                                  for p_ in range(NPAIRS - 1))
                # ... and the deferred normalizations during the last pair
                if norm_q and (ch > 0 or
                               (pair == NPAIRS - 1 and t2 >= 3)):
                    normalize_chunk(*norm_q.pop(0))
            if (pair, ch) == (NPAIRS - 1, NCH - 1):
                pv_step(ST - 4)
                pv_step(ST - 3)
                pv_step(ST - 2)
                pv_step(ST - 1)
                tail_copies()
                carry = []
            else:
                carry = [lambda p=pv_step: (p(ST - 4), p(ST - 3)),
                         lambda p=pv_step: (p(ST - 2), p(ST - 1)),
                         tail_copies]
    # --- tail: outproj for chunks 0..2 first (they don't need the last
    # chunk's normalization), the (3,3) normalize chain resolves meanwhile --
    with nc.allow_low_precision(reason="bf16 softmax scale"):
        nc.vector.reciprocal(rc_sp[96:128, :], dn_sp[96:128, :])
    for item in norm_q:  # any leftovers for pairs 0..2 / (3, ch<3)
        normalize_chunk(*item)
    norm_q.clear()

    def outproj(qt):
        # both 512-wide output halves of one q-tile share a 2-bank PSUM
        # tile: 8 matmuls, then ONE 1024-wide copy + DMA
        qs = slice(qt * 128, (qt + 1) * 128)
        po = ps_et.tile([128, 2 * CH], f32, tag="eT", name="po")
        for ot in range(EMB // CH):
            os_ = slice(ot * CH, (ot + 1) * CH)
            for pk in range(NPAIRS):
                nc.tensor.matmul(
                    po[:, os_], outu[:, pk, qs], wp_sb[:, pk, os_],
                    start=(pk == 0), stop=(pk == NPAIRS - 1),
                )
        so = stpool.tile([128, 2 * CH], f32, tag="so")
        if qt % 2 == 0:  # split tail copies across the two idle engines
            nc.scalar.copy(so[:], po[:])
        else:
            nc.vector.tensor_copy(so[:], po[:])
        nc.sync.dma_start(out[qs, :], so[:])

    for qt in range(3 * ST // 4):
        outproj(qt)
    normalize_chunk(NPAIRS - 1, NCH - 1)
    for qt in range(3 * ST // 4, ST):
        outproj(qt)


def build():
    """Build + compile the per-core Bass program (cached)."""
    if "nc" in _cache:
        return _cache["nc"]
    import concourse.tile as tile
    from concourse import bacc, mybir

    f32 = mybir.dt.float32
    bf16 = mybir.dt.bfloat16
    nc = bacc.Bacc("TRN2", target_bir_lowering=False, debug=False,
                   num_devices=NCORES)
    xdT = nc.dram_tensor("xdT", (EMB, S), bf16, kind="ExternalInput").ap()
    xeT = nc.dram_tensor("xeT", (ST, 128, EMB), bf16, kind="ExternalInput").ap()
    wqT = nc.dram_tensor("wqT", (128, ET, FG), bf16, kind="ExternalInput").ap()
    wkT = nc.dram_tensor("wkT", (128, ET, FG), bf16, kind="ExternalInput").ap()
    wvT = nc.dram_tensor("wvT", (128, ET, FG), bf16, kind="ExternalInput").ap()
    wpT = nc.dram_tensor("wpT", (128, FG // 128, EMB), bf16, kind="ExternalInput").ap()
    bq = nc.dram_tensor("bq", (FG,), f32, kind="ExternalInput").ap()
    bk = nc.dram_tensor("bk", (FG,), f32, kind="ExternalInput").ap()
    # bv pre-tiled to outu's layout: [partition 64j+d, pair]
    bv = nc.dram_tensor("bv", (128, FG // 128), f32, kind="ExternalInput").ap()
    out = nc.dram_tensor("out", (S, EMB), f32, kind="ExternalOutput").ap()

    with tile.TileContext(nc) as tc:
        with ExitStack() as ctx:
            _mha_body(tc, ctx, xdT, xeT, wqT, wkT, wvT, wpT, bq, bk, bv, out)
    nc.compile()
    _cache["nc"] = nc
    return nc


def make_in_maps(x_enc, x_dec, Wq, bq, Wk, bk, Wv, bv, Wp):
    """Host-side sharding: per-core input dict for core c = (g = c//4, b = c%4)."""
    bf = ml_dtypes.bfloat16

    def tile_w(w):     # [EMB_in, F] -> [128, ET_in, F]
        ei, f = w.shape
        return np.ascontiguousarray(
            w.reshape(ei // 128, 128, f).transpose(1, 0, 2)).astype(bf)

    in_maps = []
    xdTs = [np.ascontiguousarray(x_dec[b].T).astype(bf) for b in range(B)]
    # kt-major x_enc: xeT[kt, p, e*128 + c] = x_enc[kt*128 + c, e*128 + p]
    xeTs = []
    for b in range(B):
        blocks = x_enc[b].reshape(ST, 128, ET, 128)         # [kt, c, e, p]
        xeTs.append(np.ascontiguousarray(
            blocks.transpose(0, 3, 2, 1).reshape(ST, 128, EMB)).astype(bf))
    for c in range(NCORES):
        g, b = divmod(c, B)
        gs = slice(g * FG, (g + 1) * FG)
        in_maps.append({
            "xdT": xdTs[b],
            "xeT": xeTs[b],
            "wqT": tile_w(np.ascontiguousarray(Wq[gs].T)),
            "wkT": tile_w(np.ascontiguousarray(Wk[gs].T)),
            "wvT": tile_w(np.ascontiguousarray(Wv[gs].T)),
            "wpT": tile_w(np.ascontiguousarray(Wp[:, gs].T)),
            "bq": np.ascontiguousarray(bq[gs]).astype(np.float32),
            "bk": np.ascontiguousarray(bk[gs]).astype(np.float32),
            # [pair, j, d] -> [64j+d, pair]
            "bv": np.ascontiguousarray(
                np.asarray(bv[gs], np.float32).reshape(FG // 128, 2, 64)
                .transpose(1, 2, 0).reshape(128, FG // 128)),
        })
    return in_maps


def kernel(x_enc, x_dec, Wq, bq, Wk, bk, Wv, bv, Wp, bp):
    from concourse.bass_utils import run_bass_kernel_spmd

    x_enc = np.asarray(x_enc, dtype=np.float32)
    x_dec = np.asarray(x_dec, dtype=np.float32)
    nc = build()
    in_maps = make_in_maps(np.asarray(x_enc), np.asarray(x_dec),
                           np.asarray(Wq), np.asarray(bq), np.asarray(Wk),
                           np.asarray(bk), np.asarray(Wv), np.asarray(bv),
                           np.asarray(Wp))
    res = run_bass_kernel_spmd(nc, in_maps, core_ids=list(range(NCORES)))
    out = np.empty((B, S, EMB), dtype=np.float32)
    bp32 = np.asarray(bp, dtype=np.float32)
    for b in range(B):
        out[b] = res.results[b]["out"] + res.results[b + B]["out"] + bp32
    return out

